# revision 1
# baseline (speedup 1.0000x reference)
"""Trainium2 Bass kernel: fused MHA block (LN -> QKV -> q/k per-token LN ->
RoPE -> SDPA -> out-proj), SPMD over 8 NeuronCores.

Sharding: core c handles batch b = c//4, query-token quarter s = c%4. The host
rotates tokens (np.roll) per core so each core's query tokens are always
tokens [0, 512) of its input; attention keys/values cover all 2048 tokens
(softmax is permutation-invariant over keys). Host concatenates 8 [512, 1024]
output slices.

Per-core pipeline (all matmuls bf16 with f32 PSUM accumulation):
  P1  x-stats from e-major xT via ones-matmul (sum, sum-of-squares), rows
      processed per 512-token slab; rstd refined with one Newton step.
  P2  hT = (xT - mu)*rstd*ln_w + ln_b, e-major (per-token rows broadcast to
      128 partitions via a DRAM bounce).
  P3  Per token-tile: k/v/q projections token-major (W streamed per kind);
      v stored [tok, head, 65] with a ones column (fuses softmax denominator
      into the AV matmul); k,q get per-token LN (bn_stats) + RoPE (free-dim
      half swap, sign folded into sin table) and stream out to DRAM.
  P6  Transpose roped q,k to feature-major via XBAR dma_start_transpose.
  P7  Per head: scoresT = krT.T @ qrT (keys on partitions), exp on ScalarE
      (no max subtraction: post-LN scores are O(5)), AV matmul with ones
      column producing [65, 512] (row 64 = denominator), per-head recip
      broadcast via tiny DRAM bounce, normalized ctxT feature-major.
  P8  out = ctxT.T @ woutT token-major, DMA out.
"""

import numpy as np
import ml_dtypes

import concourse.bass as bass
import concourse.mybir as mybir
import concourse.tile as tile
from concourse import bacc
from concourse.bass_utils import run_bass_kernel_spmd

B, L, D, H, DH = 2, 2048, 1024, 16, 64
EPS = 1e-5
ROPE_BASE = 10000.0
NCORES = 8
LQ = L // 4
P = 128
ND = D // P      # 8 feature tiles of 128
NT = L // P      # 16 token tiles
NTQ = LQ // P    # 4 query token tiles
FD = 512         # psum bank free size (f32)
NSL = L // FD    # 4 slabs of keys/tokens
BF = mybir.dt.bfloat16
F32 = mybir.dt.float32
AF = mybir.ActivationFunctionType
OP = mybir.AluOpType


def _bc_part(ap, parts):
    """Partition-broadcast (step 0) of a [1, ...] DRAM AP to `parts` rows."""
    return bass.AP(tensor=ap.tensor, offset=ap.offset,
                   ap=[[0, parts]] + list(ap.ap[1:]))


def _bc_heads(ap2, n, at=1):
    """Insert a step-0 dim of size n at free position `at` of a 2D sbuf AP."""
    dims = list(ap2.ap)
    return bass.AP(tensor=ap2.tensor, offset=ap2.offset,
                   ap=dims[:at] + [[0, n]] + dims[at:])


def _emit(nc):
    xT = nc.dram_tensor("xT", [D, L], BF, kind="ExternalInput")
    wqkvT = nc.dram_tensor("wqkvT", [D, 3 * D], BF, kind="ExternalInput")
    woutT = nc.dram_tensor("woutT", [D, D], BF, kind="ExternalInput")
    ln_w = nc.dram_tensor("ln_w", [D], F32, kind="ExternalInput")
    ln_b = nc.dram_tensor("ln_b", [D], F32, kind="ExternalInput")
    q_ln_w = nc.dram_tensor("q_ln_w", [D], BF, kind="ExternalInput")
    k_ln_w = nc.dram_tensor("k_ln_w", [D], BF, kind="ExternalInput")
    cos_t = nc.dram_tensor("cos_t", [L, DH], BF, kind="ExternalInput")
    sin_t = nc.dram_tensor("sin_t", [L, DH], BF, kind="ExternalInput")
    out = nc.dram_tensor("out", [LQ, D], F32, kind="ExternalOutput")

    with tile.TileContext(nc) as tc:
        _body(nc, tc, xT, wqkvT, woutT, ln_w, ln_b, q_ln_w, k_ln_w,
              cos_t, sin_t, out)
    return nc


def _rstd_refine(nc, pool, r, vareps, shape, name):
    """One Newton step for r ~= rsqrt(varep): r' = r*(1.5 - 0.5*varep*r^2).
    Guards against ACT sqrt LUT error on hardware. In-place on r."""
    t = pool.tile(list(shape), F32, name=f"{name}_nt", tag=f"{name}_nt", bufs=2)
    nc.scalar.activation(t[:], r[:], AF.Square)
    nc.vector.tensor_mul(t[:], t[:], vareps[:])
    nc.vector.tensor_scalar(t[:], t[:], -0.5, 1.5, op0=OP.mult, op1=OP.add)
    nc.vector.tensor_mul(r[:], r[:], t[:])


def _body(nc, tc, xT, wqkvT, woutT, ln_w, ln_b, q_ln_w, k_ln_w,
          cos_t, sin_t, out):
    import contextlib
    ap_xT = xT.ap().rearrange("(nd p) t -> p nd t", p=P)
    ap_wqkvT = wqkvT.ap().rearrange("(nd p) e -> p nd e", p=P)
    ap_woutT = woutT.ap().rearrange("(nd p) e -> p nd e", p=P)
    ap_cos = cos_t.ap().rearrange("(tt p) j -> p tt j", p=P)
    ap_sin = sin_t.ap().rearrange("(tt p) j -> p tt j", p=P)

    ctx = contextlib.ExitStack()
    with ctx:
        const = ctx.enter_context(tc.tile_pool(name="const", bufs=1))
        live = ctx.enter_context(tc.tile_pool(name="live", bufs=1))
        stat = ctx.enter_context(tc.tile_pool(name="stat", bufs=1))
        dram = ctx.enter_context(tc.tile_pool(name="dram", bufs=2, space="DRAM"))

        # ---------- constants ----------
        lnw_sb = const.tile([P, ND], F32)
        nc.gpsimd.dma_start(lnw_sb[:], ln_w.ap().rearrange("(o p) -> p o", p=P))
        lnb_sb = const.tile([P, ND], F32)
        nc.gpsimd.dma_start(lnb_sb[:], ln_b.ap().rearrange("(o p) -> p o", p=P))
        qw_sb = const.tile([P, D], BF)      # q_ln_w broadcast to all partitions
        nc.gpsimd.dma_start(qw_sb[:], _bc_part(q_ln_w.ap()[None, :], P))
        kw_sb = const.tile([P, D], BF)
        nc.gpsimd.dma_start(kw_sb[:], _bc_part(k_ln_w.ap()[None, :], P))
        cos_sb = const.tile([P, NT, DH], BF)
        nc.gpsimd.dma_start(cos_sb[:], ap_cos)
        sin_sb = const.tile([P, NT, DH], BF)
        nc.gpsimd.dma_start(sin_sb[:], ap_sin)
        ones_sb = const.tile([P, 1], BF)
        nc.vector.memset(ones_sb[:], 1.0)
        wo_sb = const.tile([P, ND, D], BF)
        nc.gpsimd.dma_start(wo_sb[:], ap_woutT)

        # ---------- long-lived tensors ----------
        v_sb = live.tile([P, NT, H, DH + 1], BF)
        nc.vector.memset(v_sb[:, :, :, DH:DH + 1], 1.0)
        krT = live.tile([P, ND, L], BF)
        qrT = live.tile([P, ND, LQ], BF)
        ctxT = live.tile([P, ND, LQ], BF)

        qr_d = dram.tile([LQ, D], BF, bufs=1)
        kr_d = dram.tile([L, D], BF, bufs=1)
        rows_d = dram.tile([2, L], BF, bufs=1)

        with tc.tile_pool(name="mid", bufs=1) as mid_pool:
            hT_sb = mid_pool.tile([P, ND, L], BF)

            with tc.tile_pool(name="tmpA", bufs=1) as tmpA, \
                 tc.tile_pool(name="ps1", bufs=1, space="PSUM") as ps1:
                # ---------- P1+P2: x stats and hT, pipelined per slab ------
                for sl in range(NSL):
                    ps = ps1.tile([1, FD], F32, name="xs", tag="xs", bufs=1)
                    ps2i = ps1.tile([1, FD], F32, name="xss", tag="xss", bufs=1)
                    xsegs = []
                    for d in range(ND):
                        xseg = tmpA.tile([P, FD], BF, name="xseg", tag="xseg",
                                         bufs=10)
                        nc.sync.dma_start(xseg[:],
                                          ap_xT[:, d, sl * FD:(sl + 1) * FD])
                        xsegs.append(xseg)
                        sq = tmpA.tile([P, FD], BF, name="xsq", tag="xsq",
                                       bufs=2)
                        nc.scalar.activation(sq[:], xseg[:], AF.Square)
                        nc.tensor.matmul(ps[:], ones_sb[:], xseg[:],
                                         start=(d == 0), stop=(d == ND - 1))
                        nc.tensor.matmul(ps2i[:], ones_sb[:], sq[:],
                                         start=(d == 0), stop=(d == ND - 1))
                    # rows for this slab -> bf16 -> DRAM bounce -> broadcast
                    mean = stat.tile([1, FD], F32, name="xmean", tag="xmean")
                    nc.vector.tensor_scalar_mul(mean[:], ps[:], 1.0 / D)
                    vep = stat.tile([1, FD], F32, name="xvep", tag="xvep")
                    nc.scalar.activation(vep[:], mean[:], AF.Square)
                    nc.vector.tensor_scalar(vep[:], vep[:], -1.0, EPS,
                                            op0=OP.mult, op1=OP.add)
                    ex2 = stat.tile([1, FD], F32, name="xex2", tag="xex2")
                    nc.vector.tensor_scalar_mul(ex2[:], ps2i[:], 1.0 / D)
                    nc.vector.tensor_add(vep[:], vep[:], ex2[:])
                    r = stat.tile([1, FD], F32, name="xr", tag="xr")
                    nc.scalar.activation(r[:], vep[:], AF.Sqrt)
                    nc.vector.reciprocal(r[:], r[:])
                    _rstd_refine(nc, stat, r, vep, (1, FD), "x")
                    rbf = stat.tile([1, FD], BF, name="xrbf", tag="xrbf")
                    nc.vector.tensor_copy(rbf[:], r[:])
                    nc.sync.dma_start(rows_d[0:1, sl * FD:(sl + 1) * FD],
                                      rbf[:])
                    mrbf = stat.tile([1, FD], BF, name="xmrbf", tag="xmrbf")
                    with nc.allow_low_precision(reason="mu*r row to bf16"):
                        nc.vector.tensor_mul(mrbf[:], mean[:], r[:])
                    nc.sync.dma_start(rows_d[1:2, sl * FD:(sl + 1) * FD],
                                      mrbf[:])
                    r_b = tmpA.tile([P, FD], BF, name="r_b", tag="r_b", bufs=2)
                    nc.sync.dma_start(
                        r_b[:], _bc_part(rows_d[0:1, sl * FD:(sl + 1) * FD], P))
                    mr_b = tmpA.tile([P, FD], BF, name="mr_b", tag="mr_b",
                                     bufs=2)
                    nc.sync.dma_start(
                        mr_b[:], _bc_part(rows_d[1:2, sl * FD:(sl + 1) * FD], P))
                    # hT for this slab (reuse the xseg tiles; ln_w/ln_b on ACT)
                    for d in range(ND):
                        t1 = tmpA.tile([P, FD], BF, name="ht1", tag="ht1",
                                       bufs=2)
                        nc.vector.tensor_mul(t1[:], xsegs[d][:], r_b[:])
                        nc.vector.tensor_sub(t1[:], t1[:], mr_b[:])
                        nc.scalar.activation(hT_sb[:, d, sl * FD:(sl + 1) * FD],
                                             t1[:], AF.Identity,
                                             bias=lnb_sb[:, d:d + 1],
                                             scale=lnw_sb[:, d:d + 1])

                # ---------- P3: k,q projections + per-token LN + RoPE -----
                def project_tile(w_tile, tt):
                    pss = []
                    for sl in range(2):
                        ps = ps1.tile([P, FD], F32, name=f"pj{sl}",
                                      tag=f"pj{sl}", bufs=3)
                        for d in range(ND):
                            nc.tensor.matmul(ps[:],
                                             hT_sb[:, d, tt * P:(tt + 1) * P],
                                             w_tile[:, d, sl * FD:(sl + 1) * FD],
                                             start=(d == 0), stop=(d == ND - 1))
                        pss.append(ps)
                    return pss

                def token_ln_rope(raw, w_row, scale, tt, dst_d, name):
                    st6 = stat.tile([P, 2, 6], F32, name=f"{name}bs", tag="bs", bufs=4)
                    seg = raw[:].rearrange("p (s f) -> p s f", s=2)
                    for s2 in range(2):
                        nc.vector.bn_stats(st6[:, s2, :], seg[:, s2, :])
                    mv = stat.tile([P, 2], F32, name=f"{name}mv", tag="mv", bufs=4)
                    nc.vector.bn_aggr(mv[:], st6[:])
                    vep = stat.tile([P, 1], F32, name=f"{name}ve", tag="ve", bufs=4)
                    nc.vector.tensor_scalar(vep[:], mv[:, 1:2], 1.0, EPS,
                                            op0=OP.mult, op1=OP.add)
                    r = stat.tile([P, 1], F32, name=f"{name}r", tag="lr", bufs=4)
                    nc.scalar.activation(r[:], vep[:], AF.Sqrt)
                    nc.vector.reciprocal(r[:], r[:])
                    _rstd_refine(nc, stat, r, vep, (P, 1), "t")
                    if scale != 1.0:
                        nc.vector.tensor_scalar_mul(r[:], r[:], scale)
                    nc.vector.tensor_scalar(raw[:], raw[:], mv[:, 0:1], r[:],
                                            op0=OP.subtract, op1=OP.mult)
                    nc.gpsimd.tensor_mul(raw[:], raw[:], w_row[:])
                    xn = raw[:].rearrange("p (h j) -> p h j", j=DH)
                    t2 = tmpA.tile([P, H, DH], BF, name=f"{name}t2", tag="rp2",
                                   bufs=2)
                    nc.vector.tensor_mul(t2[:, :, 0:DH // 2],
                                         xn[:, :, DH // 2:DH],
                                         _bc_heads(sin_sb[:, tt, 0:DH // 2], H))
                    nc.vector.tensor_mul(t2[:, :, DH // 2:DH],
                                         xn[:, :, 0:DH // 2],
                                         _bc_heads(sin_sb[:, tt, DH // 2:DH], H))
                    t3 = tmpA.tile([P, H, DH], BF, name=f"{name}t3", tag="rp3",
                                   bufs=2)
                    nc.vector.tensor_mul(t3[:], xn,
                                         _bc_heads(cos_sb[:, tt, :], H))
                    nc.gpsimd.tensor_add(t3[:], t3[:], t2[:])
                    nc.sync.dma_start(dst_d[tt * P:(tt + 1) * P, :],
                                      t3[:].rearrange("p h j -> p (h j)"))

                wk_sb = mid_pool.tile([P, ND, D], BF, name="wk", tag="wslab",
                                      bufs=1)
                nc.gpsimd.dma_start(wk_sb[:], ap_wqkvT[:, :, D:2 * D])
                for tt in range(NT):
                    pss = project_tile(wk_sb, tt)
                    raw = tmpA.tile([P, D], BF, name="kraw", tag="kraw", bufs=5)
                    nc.scalar.copy(raw[:, 0:FD], pss[0][:])
                    nc.scalar.copy(raw[:, FD:D], pss[1][:])
                    token_ln_rope(raw, kw_sb, 1.0, tt, kr_d, "k")
                wq_sb = mid_pool.tile([P, ND, D], BF, name="wq", tag="wslab",
                                      bufs=1)
                nc.gpsimd.dma_start(wq_sb[:], ap_wqkvT[:, :, 0:D])
                for tt in range(NTQ):
                    pss = project_tile(wq_sb, tt)
                    raw = tmpA.tile([P, D], BF, name="qraw", tag="kraw", bufs=5)
                    nc.scalar.copy(raw[:, 0:FD], pss[0][:])
                    nc.scalar.copy(raw[:, FD:D], pss[1][:])
                    token_ln_rope(raw, qw_sb, DH ** -0.5, tt, qr_d, "q")

            # tmpA/ps1 closed. wv load + transposes + attention + v-proj.
            wv_sb = mid_pool.tile([P, ND, D], BF, name="wv", tag="wslab",
                                  bufs=1)
            nc.gpsimd.dma_start(wv_sb[:], ap_wqkvT[:, :, 2 * D:3 * D])

            with tc.tile_pool(name="tmpC", bufs=1) as tmpC, \
                 tc.tile_pool(name="ps2", bufs=1, space="PSUM") as ps2:
                nc.scalar.dma_start_transpose(qrT[:], qr_d[:])
                for sl in range(NSL):
                    nc.scalar.dma_start_transpose(
                        krT[:, :, sl * FD:(sl + 1) * FD],
                        kr_d[sl * FD:(sl + 1) * FD, :])

                # ---------- v projection (emitted first, low priority so the
                # scheduler uses it as PE filler during attention exp waits) --
                with tc.high_priority(offset=-1000000):
                    for tt in range(NT):
                        for sl in range(2):
                            ps = ps2.tile([P, FD], F32, name="vp", tag="mm2",
                                          bufs=2)
                            for d in range(ND):
                                nc.tensor.matmul(
                                    ps[:], hT_sb[:, d, tt * P:(tt + 1) * P],
                                    wv_sb[:, d, sl * FD:(sl + 1) * FD],
                                    start=(d == 0), stop=(d == ND - 1))
                            dst = v_sb[:, tt, sl * 8:(sl + 1) * 8, 0:DH]
                            nc.vector.tensor_copy(
                                dst, ps[:].rearrange("p (h e) -> p h e", e=DH))

                # ---------- P7: attention (head pairs, chunked exp) -------
                for et in range(ND):
                    hA, hB = 2 * et, 2 * et + 1
                    ctx_a = ps2.tile([DH + 1, LQ], F32, name="ctxa", tag="ctx",
                                     bufs=2)
                    ctx_b = ps2.tile([DH + 1, LQ], F32, name="ctxb", tag="ctx",
                                     bufs=2)
                    kA = krT[0:DH, et, :]
                    kB = krT[DH:P, et, :]
                    qA = qrT[0:DH, et, :]
                    qB = qrT[DH:P, et, :]
                    for g in range(NT // 2):
                        st0, st1 = 2 * g, 2 * g + 1
                        spsA = ps2.tile([P, 2, LQ], F32, name="spsA",
                                        tag="sps", bufs=2)
                        spsB = ps2.tile([P, 2, LQ], F32, name="spsB",
                                        tag="sps", bufs=2)
                        nc.tensor.matmul(spsA[:, 0, :],
                                         kA[:, st0 * P:(st0 + 1) * P], qA,
                                         start=True, stop=True)
                        nc.tensor.matmul(spsB[:, 0, :],
                                         kB[:, st0 * P:(st0 + 1) * P], qB,
                                         start=True, stop=True)
                        nc.tensor.matmul(spsA[:, 1, :],
                                         kA[:, st1 * P:(st1 + 1) * P], qA,
                                         start=True, stop=True)
                        nc.tensor.matmul(spsB[:, 1, :],
                                         kB[:, st1 * P:(st1 + 1) * P], qB,
                                         start=True, stop=True)
                        expA = tmpC.tile([P, 2, LQ], BF, name="expA",
                                         tag="exp", bufs=6)
                        expB = tmpC.tile([P, 2, LQ], BF, name="expB",
                                         tag="exp", bufs=6)
                        nc.scalar.activation(expA[:], spsA[:], AF.Exp)
                        nc.scalar.activation(expB[:], spsB[:], AF.Exp)
                        for j, st in ((0, st0), (1, st1)):
                            nc.tensor.matmul(ctx_a[:], v_sb[:, st, hA, :],
                                             expA[:, j, :],
                                             start=(st == 0),
                                             stop=(st == NT - 1))
                            nc.tensor.matmul(ctx_b[:], v_sb[:, st, hB, :],
                                             expB[:, j, :],
                                             start=(st == 0),
                                             stop=(st == NT - 1))
                    for hh, cps in ((hA, ctx_a), (hB, ctx_b)):
                        half = (hh % 2) * DH
                        rrow = stat.tile([1, LQ], BF, name="rrow", tag="rrow",
                                         bufs=2)
                        with nc.allow_low_precision(reason="softmax denom"):
                            nc.vector.reciprocal(rrow[:], cps[DH:DH + 1, :])
                        den_d = dram.tile([1, LQ], BF, name="den", bufs=2)
                        nc.sync.dma_start(den_d[:], rrow[:])
                        rb = tmpC.tile([DH, LQ], BF, name="rb", tag="rb",
                                       bufs=2)
                        nc.sync.dma_start(rb[:], _bc_part(den_d[:], DH))
                        nc.vector.tensor_mul(ctxT[half:half + DH, et, :],
                                             cps[0:DH, :], rb[:])

                # ---------- P8: output projection ----------
                for tt in range(NTQ):
                    o_sb = tmpC.tile([P, D], F32, name="osb", tag="osb",
                                     bufs=2)
                    for sl in range(2):
                        ps = ps2.tile([P, FD], F32, name="ops", tag="mm2",
                                      bufs=2)
                        for d in range(ND):
                            nc.tensor.matmul(
                                ps[:], ctxT[:, d, tt * P:(tt + 1) * P],
                                wo_sb[:, d, sl * FD:(sl + 1) * FD],
                                start=(d == 0), stop=(d == ND - 1))
                        nc.scalar.copy(o_sb[:, sl * FD:(sl + 1) * FD], ps[:])
                    nc.sync.dma_start(out.ap()[tt * P:(tt + 1) * P, :],
                                      o_sb[:])


_NC_CACHE = None


def build_nc(do_compile=True):
    nc = bacc.Bacc("TRN2", target_bir_lowering=False, debug=False)
    _emit(nc)
    if do_compile:
        nc.compile()
    return nc


def _get_nc():
    global _NC_CACHE
    if _NC_CACHE is None:
        _NC_CACHE = build_nc(do_compile=True)
    return _NC_CACHE


def _build_tables():
    inv_freq = 1.0 / (ROPE_BASE ** (np.arange(0, DH, 2, dtype=np.float32) / DH))
    t = np.arange(L, dtype=np.float32)
    freqs = np.outer(t, inv_freq)                       # [L, 32]
    cos = np.concatenate([np.cos(freqs)] * 2, axis=1)   # [L, 64]
    sin = np.concatenate([np.sin(freqs)] * 2, axis=1)
    sign = np.where(np.arange(DH) < DH // 2, -1.0, 1.0).astype(np.float32)
    return (cos.astype(ml_dtypes.bfloat16),
            (sin * sign[None, :]).astype(ml_dtypes.bfloat16))


def make_in_maps(x, ln_w, ln_b, w_qkv, q_ln_w, k_ln_w, w_out):
    wqkvT = np.ascontiguousarray(np.asarray(w_qkv, np.float32).T).astype(
        ml_dtypes.bfloat16)
    woutT = np.ascontiguousarray(np.asarray(w_out, np.float32).T).astype(
        ml_dtypes.bfloat16)
    cos_t, sin_t = _build_tables()
    x = np.asarray(x, np.float32)
    in_maps = []
    for c in range(NCORES):
        b, s = c // 4, c % 4
        xb = np.roll(x[b], -s * LQ, axis=0)
        xT = np.ascontiguousarray(xb.T).astype(ml_dtypes.bfloat16)
        in_maps.append({
            "xT": xT, "wqkvT": wqkvT, "woutT": woutT,
            "ln_w": np.asarray(ln_w, np.float32),
            "ln_b": np.asarray(ln_b, np.float32),
            "q_ln_w": np.asarray(q_ln_w, np.float32).astype(ml_dtypes.bfloat16),
            "k_ln_w": np.asarray(k_ln_w, np.float32).astype(ml_dtypes.bfloat16),
            "cos_t": np.ascontiguousarray(np.roll(cos_t, -s * LQ, axis=0)),
            "sin_t": np.ascontiguousarray(np.roll(sin_t, -s * LQ, axis=0)),
        })
    return in_maps


def kernel(x, ln_w, ln_b, w_qkv, q_ln_w, k_ln_w, w_out, **run_kwargs):
    in_maps = make_in_maps(x, ln_w, ln_b, w_qkv, q_ln_w, k_ln_w, w_out)
    nc = _get_nc()
    res = run_bass_kernel_spmd(nc, in_maps, core_ids=list(range(NCORES)),
                               **run_kwargs)
    out = np.zeros((B, L, D), np.float32)
    for c in range(NCORES):
        b, s = c // 4, c % 4
        out[b, s * LQ:(s + 1) * LQ, :] = res.results[c]["out"]
    return out



# revision 12
# speedup vs baseline: 1.6047x; 1.6047x over previous
"""Trainium2 Bass kernel: fused MHA block (LN -> QKV -> q/k per-token LN ->
RoPE -> SDPA -> out-proj), SPMD over 8 NeuronCores.

Sharding: core c handles batch b = c//4, query-token quarter s = c%4. The host
rotates tokens (np.roll) per core so each core's query tokens are always
tokens [0, 512) of its input; attention keys/values cover all 2048 tokens
(softmax is permutation-invariant over keys). Host concatenates 8 [512, 1024]
output slices.

v2 design notes (all matmuls bf16 with f32 PSUM accumulation):
  - ln_w folded into w_qkv on the host (W' = W * ln_w); ln_b enters as one
    K=1 ones-matmul accumulate (c0 = W @ ln_b) per projection psum half.
  - x normalized IN PLACE in the e-major x slab tiles (no separate hT):
    stats via ones(1/D)-matmuls; r and mu*r rows broadcast across partitions
    with K=1 matmuls into PSUM (no DRAM bounce).
  - k, v, q projections all run in phase 1 (v no longer competes with
    attention for the PE); q last reusing slab-0 xn, v slab 3 after q so the
    PE has filler during the q tail.
  - per-token q/k LN: bn_stats on DVE, affine applied on ACT via per-token
    scale/bias pointers (in place); RoPE sin-mul on DVE, cos-mul + add on
    Pool.
  - attention: scoresT = krT.T @ qrT per head pair, exp on ACT (the binding
    engine), AV with a fused ones-column producing the softmax denominator;
    denominator reciprocal broadcast via K=1 matmul (no DRAM bounce).
  - transposes (XBAR) issued from SP, weight DMAs from gpsimd, attention-
    phase PSUM evacuations on DVE, so ACT does nothing but exp there.
"""

import numpy as np
import ml_dtypes

import concourse.bass as bass
import concourse.mybir as mybir
import concourse.tile as tile
from concourse import bacc
from concourse.bass_utils import run_bass_kernel_spmd

B, L, D, H, DH = 2, 2048, 1024, 16, 64
EPS = 1e-5
ROPE_BASE = 10000.0
NCORES = 8
LQ = L // 4
P = 128
ND = D // P      # 8 feature tiles of 128
NT = L // P      # 16 token tiles
NTQ = LQ // P    # 4 query token tiles
FD = 512         # psum bank free size (f32)
NSL = L // FD    # 4 slabs of keys/tokens
BF = mybir.dt.bfloat16
F32 = mybir.dt.float32
AF = mybir.ActivationFunctionType
OP = mybir.AluOpType


def _bc_part(ap, parts):
    """Partition-broadcast (step 0) of a [1, ...] DRAM AP to `parts` rows."""
    return bass.AP(tensor=ap.tensor, offset=ap.offset,
                   ap=[[0, parts]] + list(ap.ap[1:]))


def _bc_heads(ap2, n, at=1):
    """Insert a step-0 dim of size n at free position `at` of a 2D sbuf AP."""
    dims = list(ap2.ap)
    return bass.AP(tensor=ap2.tensor, offset=ap2.offset,
                   ap=dims[:at] + [[0, n]] + dims[at:])


def _emit(nc, with_c0):
    xT = nc.dram_tensor("xT", [D, L], BF, kind="ExternalInput")
    wqkvT = nc.dram_tensor("wqkvT", [D, 3 * D], BF, kind="ExternalInput")
    woutT = nc.dram_tensor("woutT", [D, D], BF, kind="ExternalInput")
    c0_t = (nc.dram_tensor("c0_t", [3, D], BF, kind="ExternalInput")
            if with_c0 else None)
    q_ln_w = nc.dram_tensor("q_ln_w", [D], BF, kind="ExternalInput")
    k_ln_w = nc.dram_tensor("k_ln_w", [D], BF, kind="ExternalInput")
    cos_t = nc.dram_tensor("cos_t", [L, DH], BF, kind="ExternalInput")
    sin_t = nc.dram_tensor("sin_t", [L, DH], BF, kind="ExternalInput")
    out = nc.dram_tensor("out", [LQ, D], F32, kind="ExternalOutput")

    with tile.TileContext(nc) as tc:
        _body(nc, tc, xT, wqkvT, woutT, c0_t, q_ln_w, k_ln_w,
              cos_t, sin_t, out)
    return nc


def _rstd_refine(nc, pool, r, vareps, shape, name):
    """One Newton step for r ~= rsqrt(varep): r' = r*(1.5 - 0.5*varep*r^2).
    Guards against ACT sqrt LUT error on hardware. In-place on r."""
    t = pool.tile(list(shape), F32, name=f"{name}_nt", tag=f"{name}_nt", bufs=2)
    nc.scalar.activation(t[:], r[:], AF.Square)
    nc.vector.tensor_mul(t[:], t[:], vareps[:])
    nc.vector.tensor_scalar(t[:], t[:], -0.5, 1.5, op0=OP.mult, op1=OP.add)
    nc.vector.tensor_mul(r[:], r[:], t[:])


def _body(nc, tc, xT, wqkvT, woutT, c0_t, q_ln_w, k_ln_w, cos_t, sin_t, out):
    import contextlib
    ap_xT = xT.ap().rearrange("(nd p) t -> p nd t", p=P)
    ap_wqkvT = wqkvT.ap().rearrange("(nd p) e -> p nd e", p=P)
    ap_woutT = woutT.ap().rearrange("(nd p) e -> p nd e", p=P)
    ap_cos = cos_t.ap().rearrange("(tt p) j -> p tt j", p=P)
    ap_sin = sin_t.ap().rearrange("(tt p) j -> p tt j", p=P)

    ctx = contextlib.ExitStack()
    with ctx:
        const = ctx.enter_context(tc.tile_pool(name="const", bufs=1))
        wpool = ctx.enter_context(tc.tile_pool(name="wp", bufs=1))
        live = ctx.enter_context(tc.tile_pool(name="live", bufs=1))
        stat = ctx.enter_context(tc.tile_pool(name="stat", bufs=1))
        xn_pool = ctx.enter_context(tc.tile_pool(name="xn", bufs=1))
        dram = ctx.enter_context(tc.tile_pool(name="dram", bufs=1, space="DRAM"))

        # ---------- weights first (gpsimd SWDGE): wk, wv, wq; wo later -----
        wk_sb = wpool.tile([P, ND, D], BF, name="wk", tag="w1", bufs=1)
        nc.gpsimd.dma_start(wk_sb[:], ap_wqkvT[:, :, D:2 * D])
        wv_sb = wpool.tile([P, ND, D], BF, name="wv", tag="w2", bufs=1)
        nc.gpsimd.dma_start(wv_sb[:], ap_wqkvT[:, :, 2 * D:3 * D])
        wq_sb = wpool.tile([P, ND, D], BF, name="wq", tag="w3", bufs=1)
        nc.gpsimd.dma_start(wq_sb[:], ap_wqkvT[:, :, 0:D])

        # ---------- constants ----------
        qw_sb = const.tile([P, D], BF)      # q_ln_w broadcast to all partitions
        nc.gpsimd.dma_start(qw_sb[:], _bc_part(q_ln_w.ap()[None, :], P))
        kw_sb = const.tile([P, D], BF)
        nc.gpsimd.dma_start(kw_sb[:], _bc_part(k_ln_w.ap()[None, :], P))
        cos_sb = const.tile([P, NT, DH], BF)
        nc.gpsimd.dma_start(cos_sb[:], ap_cos)
        sin_sb = const.tile([P, NT, DH], BF)
        nc.gpsimd.dma_start(sin_sb[:], ap_sin)
        if c0_t is not None:
            c0_sb = const.tile([1, 3, D], BF)
            nc.gpsimd.dma_start(c0_sb[:], c0_t.ap()[None, :, :])
        onesD_sb = const.tile([P, 1], BF)    # 1/D column for the stats matmul
        nc.vector.memset(onesD_sb[:], 1.0 / D)
        ones_row = const.tile([1, P], BF)
        nc.vector.memset(ones_row[:], 1.0)

        # ---------- long-lived tensors ----------
        v_sb = live.tile([P, NT, H, DH + 1], BF)
        nc.vector.memset(v_sb[:, :, :, DH:DH + 1], 1.0)
        krT = live.tile([P, ND, L], BF)
        qrT = live.tile([P, ND, LQ], BF)
        ctxT = live.tile([P, ND, LQ], BF)

        qr_d = dram.tile([LQ, D], BF, bufs=1)
        kr_d = dram.tile([L, D], BF, bufs=1)

        xsl = []   # per-slab e-major x tiles, raw -> normalized in place

        with tc.tile_pool(name="tmpA", bufs=1) as tmpA, \
             tc.tile_pool(name="ps1", bufs=1, space="PSUM") as ps1:

            # ---------- per-tile helpers ----------
            def project_tile(w_tile, tt, kind):
                """[128 tok, 1024] projection psum pair for token tile tt.
                kind: 0=q, 1=k, 2=v (selects the folded-ln_b c0 row)."""
                sl, jt = tt // 4, tt % 4
                xn_t = xsl[sl]
                pss = []
                for s2 in range(2):
                    ps = ps1.tile([P, FD], F32, name=f"pj{s2}",
                                  tag=f"pj{s2}", bufs=2)
                    for d in range(ND):
                        nc.tensor.matmul(ps[:],
                                         xn_t[:, d, jt * P:(jt + 1) * P],
                                         w_tile[:, d, s2 * FD:(s2 + 1) * FD],
                                         start=(d == 0),
                                         stop=(c0_t is None and d == ND - 1))
                    if c0_t is not None:
                        nc.tensor.matmul(
                            ps[:], ones_row[0:1, :],
                            c0_sb[0:1, kind, s2 * FD:(s2 + 1) * FD],
                            start=False, stop=True)
                    pss.append(ps)
                return pss

            def token_ln_rope(pss, w_row, scale, tt, dst_d, name):
                raw = tmpA.tile([P, D], BF, name=f"{name}raw", tag="raw",
                                bufs=3)
                nc.scalar.copy(raw[:, 0:FD], pss[0][:])
                nc.scalar.copy(raw[:, FD:D], pss[1][:])
                st6 = stat.tile([P, 2, 6], F32, name=f"{name}bs", tag="bs",
                                bufs=4)
                seg = raw[:].rearrange("p (s f) -> p s f", s=2)
                for s2 in range(2):
                    nc.vector.bn_stats(st6[:, s2, :], seg[:, s2, :])
                mv = stat.tile([P, 2], F32, name=f"{name}mv", tag="mv", bufs=4)
                nc.vector.bn_aggr(mv[:], st6[:])
                vep = stat.tile([P, 1], F32, name=f"{name}ve", tag="ve",
                                bufs=4)
                nc.vector.tensor_scalar(vep[:], mv[:, 1:2], 1.0, EPS,
                                        op0=OP.mult, op1=OP.add)
                r = stat.tile([P, 1], F32, name=f"{name}r", tag="lr", bufs=4)
                nc.scalar.activation(r[:], vep[:], AF.Sqrt)
                nc.vector.reciprocal(r[:], r[:])
                _rstd_refine(nc, stat, r, vep, (P, 1), "t")
                if scale != 1.0:
                    nc.vector.tensor_scalar_mul(r[:], r[:], scale)
                nmb = stat.tile([P, 1], F32, name=f"{name}nmb", tag="nmb",
                                bufs=4)
                nc.vector.tensor_scalar(nmb[:], mv[:, 0:1], r[:], -1.0,
                                        op0=OP.mult, op1=OP.mult)
                # in place: raw = (raw - mu)*r on ACT (per-token ptrs), then
                # raw *= w_row on DVE
                nc.scalar.activation(raw[:], raw[:], AF.Identity,
                                     bias=nmb[:], scale=r[:])
                nc.vector.tensor_mul(raw[:], raw[:], w_row[:])
                xn = raw[:].rearrange("p (h j) -> p h j", j=DH)
                t2 = tmpA.tile([P, H, DH], BF, name=f"{name}t2", tag="rp2",
                               bufs=2)
                nc.vector.tensor_mul(t2[:, :, 0:DH // 2],
                                     xn[:, :, DH // 2:DH],
                                     _bc_heads(sin_sb[:, tt, 0:DH // 2], H))
                nc.vector.tensor_mul(t2[:, :, DH // 2:DH],
                                     xn[:, :, 0:DH // 2],
                                     _bc_heads(sin_sb[:, tt, DH // 2:DH], H))
                t3 = tmpA.tile([P, H, DH], BF, name=f"{name}t3", tag="rp3",
                               bufs=2)
                nc.gpsimd.tensor_mul(t3[:], xn,
                                     _bc_heads(cos_sb[:, tt, :], H))
                nc.gpsimd.tensor_add(t3[:], t3[:], t2[:])
                nc.sync.dma_start(dst_d[tt * P:(tt + 1) * P, :],
                                  t3[:].rearrange("p h j -> p (h j)"))

            def v_tile(tt):
                pss = project_tile(wv_sb, tt, 2)
                for s2 in range(2):
                    dst = v_sb[:, tt, s2 * 8:(s2 + 1) * 8, 0:DH]
                    nc.vector.tensor_copy(
                        dst, pss[s2][:].rearrange("p (h e) -> p h e", e=DH))

            # ---------- phase 1: stats + in-place LN + k/v projections -----
            for sl in range(NSL):
                tag = "xsl0" if sl == 0 else "xslR"
                xt = xn_pool.tile([P, ND, FD], BF, name=f"xsl{sl}",
                                  tag=tag, bufs=(1 if sl == 0 else 2))
                xsl.append(xt)
                nc.sync.dma_start(xt[:], ap_xT[:, :, sl * FD:(sl + 1) * FD])
                ps_s = ps1.tile([1, FD], F32, name="xs", tag="xs", bufs=1)
                ps_q = ps1.tile([1, FD], F32, name="xss", tag="xss", bufs=1)
                for d in range(ND):
                    sq = tmpA.tile([P, FD], BF, name="xsq", tag="xsq", bufs=2)
                    nc.scalar.activation(sq[:], xt[:, d, :], AF.Square)
                    nc.tensor.matmul(ps_s[:], onesD_sb[:], xt[:, d, :],
                                     start=(d == 0), stop=(d == ND - 1))
                    nc.tensor.matmul(ps_q[:], onesD_sb[:], sq[:],
                                     start=(d == 0), stop=(d == ND - 1))
                # ps_s = mean, ps_q = E[x^2] (ones column carries 1/D)
                vep = stat.tile([1, FD], F32, name="xvep", tag="xvep")
                nc.scalar.activation(vep[:], ps_s[:], AF.Square)
                nc.vector.tensor_scalar(vep[:], vep[:], -1.0, EPS,
                                        op0=OP.mult, op1=OP.add)
                nc.vector.scalar_tensor_tensor(vep[:], ps_q[:], 1.0, vep[:],
                                               op0=OP.mult, op1=OP.add)
                r = stat.tile([1, FD], F32, name="xr", tag="xr")
                nc.scalar.activation(r[:], vep[:], AF.Sqrt)
                nc.vector.reciprocal(r[:], r[:])
                _rstd_refine(nc, stat, r, vep, (1, FD), "x")
                rows = stat.tile([1, 2, FD], BF, name="xrows", tag="xrows",
                                 bufs=2)
                nc.vector.tensor_copy(rows[:, 0, :], r[:])
                with nc.allow_low_precision(reason="mu*r row to bf16"):
                    nc.vector.tensor_mul(rows[:, 1, :], ps_s[:], r[:])
                # broadcast r and mu*r across partitions via K=1 matmuls
                bc_ps = ps1.tile([P, 2, FD], F32, name="bc", tag="bc", bufs=1)
                nc.tensor.matmul(bc_ps[:, 0, :], ones_row[0:1, :],
                                 rows[:, 0, :], start=True, stop=True)
                nc.tensor.matmul(bc_ps[:, 1, :], ones_row[0:1, :],
                                 rows[:, 1, :], start=True, stop=True)
                rbmr = tmpA.tile([P, 2, FD], BF, name="rbmr", tag="rbmr",
                                 bufs=2)
                nc.scalar.copy(rbmr[:], bc_ps[:])
                # in-place: x <- x*r - mu*r  (e-major)
                for d in range(ND):
                    nc.vector.tensor_mul(xt[:, d, :], xt[:, d, :],
                                         rbmr[:, 0, :])
                    nc.gpsimd.tensor_sub(xt[:, d, :], xt[:, d, :],
                                         rbmr[:, 1, :])
                # k tiles of this slab, then its transpose; v tiles (slab 3's
                # deferred past q so the PE has filler during the q tail)
                for tt in range(sl * 4, sl * 4 + 4):
                    pss = project_tile(wk_sb, tt, 1)
                    token_ln_rope(pss, kw_sb, 1.0, tt, kr_d, "k")
                nc.sync.dma_start_transpose(
                    krT[:, :, sl * FD:(sl + 1) * FD],
                    kr_d[sl * FD:(sl + 1) * FD, :])
                if sl < NSL - 1:
                    for tt in range(sl * 4, sl * 4 + 4):
                        v_tile(tt)

            # ---------- q tiles (slab 0 xn still resident), then v slab 3 --
            for tt in range(NTQ):
                pss = project_tile(wq_sb, tt, 0)
                token_ln_rope(pss, qw_sb, DH ** -0.5, tt, qr_d, "q")
            nc.sync.dma_start_transpose(qrT[:], qr_d[:])
            for tt in range(12, 16):
                v_tile(tt)

        # wq slot done; load w_out for the final projection (SWDGE).
        wo_sb = wpool.tile([P, ND, D], BF, name="wo", tag="w3", bufs=1)
        nc.gpsimd.dma_start(wo_sb[:], ap_woutT)

        with tc.tile_pool(name="tmpC", bufs=1) as tmpC, \
             tc.tile_pool(name="ps2", bufs=1, space="PSUM") as ps2:
            # ---------- attention (head pairs, chunked exp) -------
            for et in range(ND):
                hA, hB = 2 * et, 2 * et + 1
                ctx_a = ps2.tile([DH + 1, LQ], F32, name="ctxa", tag="ctx",
                                 bufs=2)
                ctx_b = ps2.tile([DH + 1, LQ], F32, name="ctxb", tag="ctx",
                                 bufs=2)
                kA = krT[0:DH, et, :]
                kB = krT[DH:P, et, :]
                qA = qrT[0:DH, et, :]
                qB = qrT[DH:P, et, :]
                for g in range(NT // 2):
                    st0, st1 = 2 * g, 2 * g + 1
                    spsA = ps2.tile([P, 2, LQ], F32, name="spsA",
                                    tag="sps", bufs=2)
                    spsB = ps2.tile([P, 2, LQ], F32, name="spsB",
                                    tag="sps", bufs=2)
                    nc.tensor.matmul(spsA[:, 0, :],
                                     kA[:, st0 * P:(st0 + 1) * P], qA,
                                     start=True, stop=True)
                    nc.tensor.matmul(spsB[:, 0, :],
                                     kB[:, st0 * P:(st0 + 1) * P], qB,
                                     start=True, stop=True)
                    nc.tensor.matmul(spsA[:, 1, :],
                                     kA[:, st1 * P:(st1 + 1) * P], qA,
                                     start=True, stop=True)
                    nc.tensor.matmul(spsB[:, 1, :],
                                     kB[:, st1 * P:(st1 + 1) * P], qB,
                                     start=True, stop=True)
                    expA = tmpC.tile([P, 2, LQ], BF, name="expA",
                                     tag="exp", bufs=5)
                    expB = tmpC.tile([P, 2, LQ], BF, name="expB",
                                     tag="exp", bufs=5)
                    nc.scalar.activation(expA[:], spsA[:], AF.Exp)
                    nc.scalar.activation(expB[:], spsB[:], AF.Exp)
                    for j, st in ((0, st0), (1, st1)):
                        nc.tensor.matmul(ctx_a[:], v_sb[:, st, hA, :],
                                         expA[:, j, :],
                                         start=(st == 0),
                                         stop=(st == NT - 1))
                        nc.tensor.matmul(ctx_b[:], v_sb[:, st, hB, :],
                                         expB[:, j, :],
                                         start=(st == 0),
                                         stop=(st == NT - 1))
                for hh, cps in ((hA, ctx_a), (hB, ctx_b)):
                    half = (hh % 2) * DH
                    rrow = stat.tile([1, LQ], BF, name="rrow", tag="rrow",
                                     bufs=2)
                    with nc.allow_low_precision(reason="softmax denom"):
                        nc.vector.reciprocal(rrow[:], cps[DH:DH + 1, :])
                    rb_ps = ps2.tile([DH, LQ], F32, name="rbps", tag="rbps",
                                     bufs=2)
                    nc.tensor.matmul(rb_ps[:], ones_row[0:1, 0:DH], rrow[:],
                                     start=True, stop=True)
                    rb = tmpC.tile([DH, LQ], BF, name="rb", tag="rb",
                                   bufs=2)
                    nc.vector.tensor_copy(rb[:], rb_ps[:])
                    nc.vector.tensor_mul(ctxT[half:half + DH, et, :],
                                         cps[0:DH, :], rb[:])

            # ---------- output projection ----------
            for tt in range(NTQ):
                o_sb = tmpC.tile([P, D], F32, name="osb", tag="osb",
                                 bufs=2)
                for s2 in range(2):
                    ps = ps2.tile([P, FD], F32, name="ops", tag="rbps",
                                  bufs=2)
                    for d in range(ND):
                        nc.tensor.matmul(
                            ps[:], ctxT[:, d, tt * P:(tt + 1) * P],
                            wo_sb[:, d, s2 * FD:(s2 + 1) * FD],
                            start=(d == 0), stop=(d == ND - 1))
                    nc.vector.tensor_copy(o_sb[:, s2 * FD:(s2 + 1) * FD],
                                          ps[:])
                nc.sync.dma_start(out.ap()[tt * P:(tt + 1) * P, :],
                                  o_sb[:])


_NC_CACHE = {}


def build_nc(do_compile=True, with_c0=False):
    nc = bacc.Bacc("TRN2", target_bir_lowering=False, debug=False)
    _emit(nc, with_c0)
    if do_compile:
        nc.compile()
    return nc


def _get_nc(with_c0=False):
    if with_c0 not in _NC_CACHE:
        _NC_CACHE[with_c0] = build_nc(do_compile=True, with_c0=with_c0)
    return _NC_CACHE[with_c0]


def _build_tables():
    inv_freq = 1.0 / (ROPE_BASE ** (np.arange(0, DH, 2, dtype=np.float32) / DH))
    t = np.arange(L, dtype=np.float32)
    freqs = np.outer(t, inv_freq)                       # [L, 32]
    cos = np.concatenate([np.cos(freqs)] * 2, axis=1)   # [L, 64]
    sin = np.concatenate([np.sin(freqs)] * 2, axis=1)
    sign = np.where(np.arange(DH) < DH // 2, -1.0, 1.0).astype(np.float32)
    return (cos.astype(ml_dtypes.bfloat16),
            (sin * sign[None, :]).astype(ml_dtypes.bfloat16))


def make_in_maps(x, ln_w, ln_b, w_qkv, q_ln_w, k_ln_w, w_out):
    w_qkv = np.asarray(w_qkv, np.float32)
    ln_w = np.asarray(ln_w, np.float32)
    ln_b = np.asarray(ln_b, np.float32)
    # fold the x-layernorm affine into the projection (exact):
    #   qkv = ((x-mu)*r * ln_w + ln_b) @ W^T
    #       = ((x-mu)*r) @ (W*ln_w)^T + (W @ ln_b)
    wf = w_qkv * ln_w[None, :]
    c0 = (w_qkv @ ln_b).reshape(3, D)
    with_c0 = bool(np.any(c0 != 0.0))
    wqkvT = np.ascontiguousarray(wf.T).astype(ml_dtypes.bfloat16)
    woutT = np.ascontiguousarray(np.asarray(w_out, np.float32).T).astype(
        ml_dtypes.bfloat16)
    cos_t, sin_t = _build_tables()
    x = np.asarray(x, np.float32)
    in_maps = []
    for c in range(NCORES):
        b, s = c // 4, c % 4
        xb = np.roll(x[b], -s * LQ, axis=0)
        xT = np.ascontiguousarray(xb.T).astype(ml_dtypes.bfloat16)
        im = {
            "xT": xT, "wqkvT": wqkvT, "woutT": woutT,
            "q_ln_w": np.asarray(q_ln_w, np.float32).astype(ml_dtypes.bfloat16),
            "k_ln_w": np.asarray(k_ln_w, np.float32).astype(ml_dtypes.bfloat16),
            "cos_t": np.ascontiguousarray(np.roll(cos_t, -s * LQ, axis=0)),
            "sin_t": np.ascontiguousarray(np.roll(sin_t, -s * LQ, axis=0)),
        }
        if with_c0:
            im["c0_t"] = c0.astype(ml_dtypes.bfloat16)
        in_maps.append(im)
    return in_maps, with_c0


def kernel(x, ln_w, ln_b, w_qkv, q_ln_w, k_ln_w, w_out, **run_kwargs):
    in_maps, with_c0 = make_in_maps(x, ln_w, ln_b, w_qkv, q_ln_w, k_ln_w,
                                    w_out)
    nc = _get_nc(with_c0)
    res = run_bass_kernel_spmd(nc, in_maps, core_ids=list(range(NCORES)),
                               **run_kwargs)
    out = np.zeros((B, L, D), np.float32)
    for c in range(NCORES):
        b, s = c // 4, c % 4
        out[b, s * LQ:(s + 1) * LQ, :] = res.results[c]["out"]
    return out
